# revision 17
# baseline (speedup 1.0000x reference)
"""Bahdanau-style attention kernel for Trainium2, data-parallel over batch on
8 NeuronCores, specialized at compile time to the src_length distribution.

Reference computation (per batch b):
    enc   = enc_state @ W_enc.T            # [S, H]
    dec   = W_dec @ dec_state              # [H]
    t     = tanh(enc + dec)                # [S, H]
    en    = t @ W_energy.T                 # [S]
    en    = where(arange(S) < L, en, -inf)
    alpha = softmax(en)                    # [S]
    ctx   = alpha @ enc_state              # [2H]

Only the first L_b columns contribute (alphas are exactly 0 beyond L_b), so
the kernel processes ceil-to-128 column budgets instead of the full S=2048.
Because the program is SPMD across the 8 cores, per-batch trip counts must be
core-uniform: batches are sorted by length and dealt into 4 slots x 8 cores;
slot k's trip count T_k is the rounded max length within the slot, baked into
the program at build time (programs are cached per trips tuple, so a new
src_length distribution recompiles once).

Single fused pass per slot, blocked over 512 columns:
  - Projection on TensorE in bf16 (1 cycle/row, halved DMA): stationary
    W_encT tiles x moving transposed-enc slabs -> PSUM [h, s-block].
  - ScalarE tanh fused with the +dec per-partition bias, output bf16.
  - Energy reduction over h as M=1 matmuls (pipelined one ht behind tanh so
    the PE never waits on ScalarE).
  - Softmax WITHOUT max-subtraction: |energy| <= ||W_energy||_1 ~ 26, so
    exp cannot overflow fp32. Masking is an additive -1e30 precomputed row.
    exp's accum_out yields per-block partial sums of the denominator.
  - The exp row is transposed to columns via PE transpose and the context
    accumulates in PSUM across blocks (M=1 matmuls over natural-layout
    enc slabs, also bf16), overlapped with the next block's projection.
  - Per slot finale: z = sum of partials, reciprocal, scale the alpha row
    and the context row, DMA out. Alphas beyond the trip count rely on the
    runner's zero-initialized output buffers.
"""

import numpy as np

import concourse.tile as tile
from concourse import bacc, mybir
from concourse.bass_utils import run_bass_kernel_spmd

B, S, H = 32, 2048, 1024
E = 2 * H
NCORES = 8
SLOTS = B // NCORES
P = 128
SBLK = 512
NEG = np.float32(-1e30)

f32 = mybir.dt.float32
f32r = mybir.dt.float32r
bf16 = mybir.dt.bfloat16
AF = mybir.ActivationFunctionType
FULL_TRIPS = (S,) * SLOTS


def _blocks(trip):
    """Split a column budget into blocks of <= SBLK (last one may be any size)."""
    out = []
    off = 0
    while off < trip:
        w = min(SBLK, trip - off)
        out.append((off, w))
        off += w
    return out


def build_program(trips=None, n_iter=1, n_cores=NCORES):
    if trips is None:
        trips = _LAST_TRIPS
    trips = tuple(int(t) for t in trips)
    assert len(trips) == SLOTS and all(P <= t <= S and t % P == 0 for t in trips)

    ET = E // P   # contraction tiles of the projection
    HT = H // P   # h tiles
    DT = H // P   # d tiles of the dec projection
    EJ = E // SBLK  # 512-wide output chunks of the context row

    nc = bacc.Bacc(
        "TRN2", target_bir_lowering=False, debug=False, num_devices=n_cores
    )
    encT_d = [
        nc.dram_tensor(f"encT{k}", [E, t], bf16, kind="ExternalInput")
        for k, t in enumerate(trips)
    ]
    encN_d = [
        nc.dram_tensor(f"encN{k}", [t, E], bf16, kind="ExternalInput")
        for k, t in enumerate(trips)
    ]
    wencT_d = nc.dram_tensor("wencT", [E, H], bf16, kind="ExternalInput")
    wdecT_d = nc.dram_tensor("wdecT", [H, H], f32r, kind="ExternalInput")
    # decm[p, ht*SLOTS + k] = dec[k, ht*128 + p]; wem[p, ht] = W_energy[ht*128+p]
    dec_d = nc.dram_tensor("decm", [P, HT * SLOTS], f32r, kind="ExternalInput")
    we_d = nc.dram_tensor("wem", [P, HT], bf16, kind="ExternalInput")
    amask_d = nc.dram_tensor("amask", [SLOTS, S], f32, kind="ExternalInput")
    ctx_d = nc.dram_tensor("ctx", [SLOTS, E], f32, kind="ExternalOutput")
    alph_d = nc.dram_tensor("alph", [SLOTS, S], f32, kind="ExternalOutput")

    with tile.TileContext(nc) as tc:
        with tc.tile_pool(name="persist", bufs=1) as persist:
            wenc_sb = persist.tile([P, ET, H], bf16)
            nc.sync.dma_start(
                out=wenc_sb[:],
                in_=wencT_d.rearrange("(et p) h -> p et h", p=P),
            )
            we_sb = persist.tile([P, HT], bf16)
            nc.sync.dma_start(out=we_sb[:], in_=we_d[:, :])
            one_sb = persist.tile([1, 1], f32)
            nc.vector.memset(one_sb, 1.0)
            decp_sb = persist.tile([P, HT * SLOTS], f32)

            # dec projection: decp[h, k] = sum_d W_dec[h, d] * dec[k, d]
            with tc.tile_pool(name="decw", bufs=1) as decw, tc.tile_pool(
                name="decps", bufs=1, space="PSUM"
            ) as decps:
                wdec_sb = decw.tile([P, DT, H], f32r)
                nc.sync.dma_start(
                    out=wdec_sb[:],
                    in_=wdecT_d.rearrange("(dt p) h -> p dt h", p=P),
                )
                dec_sb = decw.tile([P, DT * SLOTS], f32r)
                nc.sync.dma_start(out=dec_sb[:], in_=dec_d[:, :])
                psd = decps.tile([P, HT * SLOTS], f32)
                for ht in range(HT):
                    for dt in range(DT):
                        nc.tensor.matmul(
                            psd[:, ht * SLOTS : (ht + 1) * SLOTS],
                            wdec_sb[:, dt, ht * P : (ht + 1) * P],
                            dec_sb[:, dt * SLOTS : (dt + 1) * SLOTS],
                            start=(dt == 0),
                            stop=(dt == DT - 1),
                        )
                nc.vector.tensor_copy(decp_sb[:], psd[:])

            with (
                tc.tile_pool(name="te", bufs=2) as te_pool,
                tc.tile_pool(name="cn", bufs=2) as cn_pool,
                tc.tile_pool(name="tt", bufs=4) as tt_pool,
                tc.tile_pool(name="rows", bufs=2) as rows,
                tc.tile_pool(name="small", bufs=2) as small,
                tc.tile_pool(name="pp", bufs=2, space="PSUM") as pp,
                tc.tile_pool(name="pe", bufs=1, space="PSUM") as pe,
                tc.tile_pool(name="pa", bufs=1, space="PSUM") as pa,
                tc.tile_pool(name="pc", bufs=1, space="PSUM") as pc,
            ):
                # Per-slot live state, re-created each (iter, slot).
                state = {}

                def emit_slot_open(k):
                    trip = trips[k]
                    st = {
                        "k": k,
                        "trip": trip,
                        "nb": len(_blocks(trip)),
                        "amrow": rows.tile([1, S], f32, tag="amrow", name="amrow"),
                        "arow": rows.tile([1, S], f32, tag="arow", name="arow"),
                        "acol": rows.tile(
                            [P, (S + P - 1) // P], bf16, tag="acol", name="acol"
                        ),
                        "zp": small.tile([1, SBLK // P], f32, tag="zp", name="zp"),
                        "pctx": pc.tile([1, E], f32, tag="pctx", name="pctx"),
                    }
                    nc.sync.dma_start(
                        out=st["amrow"][0:1, 0:trip], in_=amask_d[k, 0:trip]
                    )
                    return st

                def emit_proj(st, j, off, w):
                    """Projection + energy + mask + exp for one block.

                    Returns the natural-layout enc slab for the post phase.
                    """
                    k = st["k"]
                    te = te_pool.tile([P, ET, SBLK], bf16, tag="te")
                    nc.sync.dma_start(
                        out=te[:, :, 0:w],
                        in_=encT_d[k]
                        .rearrange("(et p) s -> p et s", p=P)[:, :, off : off + w],
                    )
                    cnt = w // P
                    cn = cn_pool.tile([P, SBLK // P, E], bf16, tag="cn")
                    nc.sync.dma_start(
                        out=cn[:, 0:cnt, :],
                        in_=encN_d[k]
                        .rearrange("(c p) e -> p c e", p=P)[
                            :, off // P : off // P + cnt, :
                        ],
                    )
                    pet = pe.tile([1, SBLK], f32, tag="pet")
                    prev_tt = None
                    for ht in range(HT):
                        ppt = pp.tile([P, SBLK], f32, tag="ppt")
                        for et in range(ET):
                            nc.tensor.matmul(
                                ppt[:, 0:w],
                                wenc_sb[:, et, ht * P : (ht + 1) * P],
                                te[:, et, 0:w],
                                start=(et == 0),
                                stop=(et == ET - 1),
                            )
                        tt = tt_pool.tile([P, SBLK], bf16, tag="tt")
                        nc.scalar.activation(
                            tt[:, 0:w],
                            ppt[:, 0:w],
                            AF.Tanh,
                            bias=decp_sb[:, ht * SLOTS + k : ht * SLOTS + k + 1],
                        )
                        if prev_tt is not None:
                            nc.tensor.matmul(
                                pet[0:1, 0:w],
                                we_sb[:, ht - 1 : ht],
                                prev_tt[:, 0:w],
                                start=(ht == 1),
                                stop=False,
                            )
                        prev_tt = tt
                    nc.tensor.matmul(
                        pet[0:1, 0:w],
                        we_sb[:, HT - 1 : HT],
                        prev_tt[:, 0:w],
                        start=False,
                        stop=True,
                    )
                    erow = small.tile([1, SBLK], f32, tag="erow")
                    nc.vector.tensor_add(
                        erow[0:1, 0:w],
                        pet[0:1, 0:w],
                        st["amrow"][0:1, off : off + w],
                    )
                    nc.scalar.activation(
                        st["arow"][0:1, off : off + w],
                        erow[0:1, 0:w],
                        AF.Exp,
                        accum_out=st["zp"][0:1, j : j + 1],
                    )
                    return cn

                def emit_post(st, cn, j, off, w):
                    """Transpose the exp row; accumulate context (PE)."""
                    trip = st["trip"]
                    cnt = w // P
                    pat = pa.tile([P, SBLK // P], f32, tag="pat")
                    for c in range(cnt):
                        nc.tensor.matmul(
                            pat[:, c : c + 1],
                            st["arow"][0:1, off + c * P : off + (c + 1) * P],
                            one_sb[:],
                            is_transpose=True,
                            start=True,
                            stop=True,
                        )
                    nc.vector.tensor_copy(
                        st["acol"][:, off // P : off // P + cnt], pat[:, 0:cnt]
                    )
                    last_c = trip // P - 1
                    for c in range(cnt):
                        gc = off // P + c
                        for jj in range(EJ):
                            nc.tensor.matmul(
                                st["pctx"][0:1, jj * SBLK : (jj + 1) * SBLK],
                                st["acol"][:, gc : gc + 1],
                                cn[:, c, jj * SBLK : (jj + 1) * SBLK],
                                start=(gc == 0),
                                stop=(gc == last_c),
                            )

                def emit_final(st):
                    k = st["k"]
                    trip = st["trip"]
                    z = small.tile([1, 1], f32, tag="z")
                    nc.vector.reduce_sum(
                        z[:], st["zp"][0:1, 0 : st["nb"]], axis=mybir.AxisListType.X
                    )
                    rz = small.tile([1, 1], f32, tag="rz")
                    nc.vector.reciprocal(rz[:], z[:])
                    anrow = rows.tile([1, S], f32, tag="anrow")
                    nc.vector.tensor_scalar_mul(
                        anrow[0:1, 0:trip], st["arow"][0:1, 0:trip], rz[0:1, 0:1]
                    )
                    nc.sync.dma_start(out=alph_d[k, 0:trip], in_=anrow[0:1, 0:trip])
                    crow = rows.tile([1, E], f32, tag="crow")
                    nc.scalar.activation(
                        crow[:], st["pctx"][:], AF.Copy, scale=rz[0:1, 0:1]
                    )
                    nc.sync.dma_start(out=ctx_d[k], in_=crow[:])

                # Flat work-item stream, software-pipelined by one block:
                # PE order is proj(item i) then post(item i-1), so transposes
                # and context matmuls always have their inputs ready.
                items = []
                for _ in range(n_iter):
                    for k in range(SLOTS):
                        for j, (off, w) in enumerate(_blocks(trips[k])):
                            items.append((k, j, off, w))
                prev = None
                for it in items:
                    k, j, off, w = it
                    if j == 0:
                        state[k] = emit_slot_open(k)
                    st = state[k]
                    cn = emit_proj(st, j, off, w)
                    if prev is not None:
                        pst, pcn, pj, poff, pw = prev
                        emit_post(pst, pcn, pj, poff, pw)
                        if pj == pst["nb"] - 1:
                            emit_final(pst)
                    prev = (st, cn, j, off, w)
                if prev is not None:
                    pst, pcn, pj, poff, pw = prev
                    emit_post(pst, pcn, pj, poff, pw)
                    emit_final(pst)
    nc.compile()
    return nc


_progs = {}
_LAST_TRIPS = FULL_TRIPS


def _get_prog(trips=None, n_iter=1):
    if trips is None:
        trips = _LAST_TRIPS
    key = (tuple(trips), n_iter)
    if key not in _progs:
        _progs[key] = build_program(trips=trips, n_iter=n_iter)
    return _progs[key]


def _assignment(src_length):
    """Sort batches by rounded length, deal rank r to slot r//8, core r%8.
    Trips stay multiples of 128: partial-partition transposes/matmuls from
    exact trips pass CoreSim and walrus but wedge the exec unit on real HW
    (NRT_EXEC_UNIT_UNRECOVERABLE), so don't go below 128 granularity."""
    L = np.asarray(src_length).astype(np.int64)
    Lr = np.minimum(np.maximum((L + P - 1) // P, 1) * P, S)
    order = np.argsort(-Lr, kind="stable")
    trips = tuple(int(Lr[order[k * NCORES]]) for k in range(SLOTS))
    return order, trips


def _build_in_maps(dec_state, enc_state, src_length, W_enc, W_dec, W_energy):
    import ml_dtypes

    bft = ml_dtypes.bfloat16
    order, trips = _assignment(src_length)
    wencT = np.ascontiguousarray(W_enc.T.astype(bft))
    wdecT = np.ascontiguousarray(W_dec.T)
    wEm = np.ascontiguousarray(W_energy[0].reshape(HT_, P).T.astype(bft))
    iota = np.arange(S, dtype=np.int64)
    L = np.asarray(src_length).astype(np.int64)

    enc_b = enc_state.astype(bft)  # [B, S, E] bf16 once

    in_maps = []
    for c in range(NCORES):
        gs = [int(order[k * NCORES + c]) for k in range(SLOTS)]
        m = {}
        for k, g in enumerate(gs):
            t = trips[k]
            m[f"encT{k}"] = np.ascontiguousarray(enc_b[g, 0:t, :].T)
            m[f"encN{k}"] = np.ascontiguousarray(enc_b[g, 0:t, :])
        lens = L[gs]
        m["amask"] = np.where(
            iota[None, :] < lens[:, None], np.float32(0.0), NEG
        ).astype(np.float32)
        # decm[p, ht*SLOTS + k] = dec_state[g_k, 0, ht*128 + p]
        m["decm"] = np.ascontiguousarray(
            dec_state[gs, 0, :].reshape(SLOTS, HT_, P).transpose(2, 1, 0)
            .reshape(P, HT_ * SLOTS)
        ).astype(np.float32)
        m["wencT"] = wencT
        m["wdecT"] = wdecT.astype(np.float32)
        m["wem"] = wEm
        in_maps.append(m)
    return in_maps, order, trips


HT_ = H // P


def _prepare_in_maps(inputs):
    maps, _, _ = _build_in_maps(
        np.asarray(inputs["dec_state"], dtype=np.float32),
        np.asarray(inputs["enc_state"], dtype=np.float32),
        np.asarray(inputs["src_length"]),
        np.asarray(inputs["W_enc"], dtype=np.float32),
        np.asarray(inputs["W_dec"], dtype=np.float32),
        np.asarray(inputs["W_energy"], dtype=np.float32),
    )
    return maps


def kernel(dec_state, enc_state, src_length, W_enc, W_dec, W_energy):
    global _LAST_TRIPS
    in_maps, order, trips = _build_in_maps(
        np.asarray(dec_state, dtype=np.float32),
        np.asarray(enc_state, dtype=np.float32),
        np.asarray(src_length),
        np.asarray(W_enc, dtype=np.float32),
        np.asarray(W_dec, dtype=np.float32),
        np.asarray(W_energy, dtype=np.float32),
    )
    _LAST_TRIPS = trips
    nc = _get_prog(trips)
    try:
        res = run_bass_kernel_spmd(nc, in_maps, list(range(NCORES)))
    except Exception:
        res = run_bass_kernel_spmd(nc, in_maps, list(range(NCORES)))
    ctx = np.zeros((B, 1, E), np.float32)
    alph = np.zeros((B, 1, S), np.float32)
    for c in range(NCORES):
        for k in range(SLOTS):
            g = int(order[k * NCORES + c])
            ctx[g, 0, :] = res.results[c]["ctx"][k]
            alph[g, 0, :] = res.results[c]["alph"][k]
    return ctx, alph
